# revision 15
# baseline (speedup 1.0000x reference)
"""Trainium2 Bass kernel for nn_Attention (dense transformer attention block).

Reference computation (shapes hardcoded):
  x [1, 256, 64, 64]; 1x1 conv+BN to q/k/v; 8 heads; per-head
  softmax(q @ k) @ v over n=4096 positions (dqk=32, dv=128); 1x1 conv+BN proj.

Sharding: one head per NeuronCore (8 cores). Each core computes its head's
attention plus its partial contribution to the projection conv
(z_h = Wp_h @ out_h); the host sums the 8 partials (the only cross-head
coupling is the channel-concat feeding the proj conv, which is a sum of
per-head matmuls).

Folding tricks (host-side, exact):
 - BN scales sq/sk fold into wq/wk rows; biases tq/tk applied on-device
   per-partition.
 - v's BN affine (sv, tv) folds through the attention (softmax rows sum to 1):
   sv scales wp columns, tv contributes a constant bias via wp @ tv.
 - proj BN affine (sp, tp) folds into wp rows / the constant bias.

On-device layout avoids all transposes:
 - QT = [32, n] and K = [32, n] conv outputs (fp16), replicated 2x along
   partitions so S^T tiles come from 2-way row-packed (tile_position) matmuls.
 - S^T duo blocks [128 m, 2, 512 q] in PSUM -> one exp (ScalarE) -> E fp16.
 - V computed directly transposed ([m, dv] blocks, fp16) via lhsT=x.
 - PV accumulates over m in PSUM; softmax denominator via col-packed
   ones-matmul (colsum) of E; normalization + projection per q-chunk,
   fully software-pipelined (depth 2) so ScalarE exp runs back-to-back.

Matmul dtypes: float32r (full-rate fp32) for the q/k convs and the proj;
fp16 for S^T / PV / colsum / V. End-to-end absmax error vs fp64 reference
is ~1e-3 relative to output scale.
"""

import numpy as np

import concourse.bacc as bacc
import concourse.mybir as mybir
import concourse.tile as tile
from concourse.bass_utils import run_bass_kernel_spmd

F32 = mybir.dt.float32
F32R = mybir.dt.float32r
FP16 = mybir.dt.float16
EXP = mybir.ActivationFunctionType.Exp

N = 4096          # positions (64*64)
C = 256           # input channels
H = 8             # heads
DQK = 32
DV = 128
NMB = N // 128    # 32 m-blocks
NQC = 8           # q-chunks of 512
QC = 512

_CACHED = {}


def _build_program():
    nc = bacc.Bacc(trn_type="TRN2")

    x2 = nc.dram_tensor("x2", [128, 2, N], F32R, kind="ExternalInput")
    x2h = nc.dram_tensor("x2h", [128, 2, N], FP16, kind="ExternalInput")
    # all fp32 weights/biases packed into one tensor -> one startup DMA:
    # cols 0:256 wq (2 chunks), 256:512 wk, 512:768 wp, 768 tq, 769 tk,
    # 770:772 bias
    wcat = nc.dram_tensor("wcat", [128, 772], F32R, kind="ExternalInput")
    wv_r = nc.dram_tensor("wv_r", [128, 2, 128], FP16, kind="ExternalInput")
    z = nc.dram_tensor("z", [2, 128, N], F32, kind="ExternalOutput")

    # x pieces: small first piece so the first conv starts ASAP
    xsplits = [(0, 512), (512, 1536), (1536, 2560), (2560, 4096)]

    with tile.TileContext(nc) as tc:
        with (
            tc.tile_pool(name="wgt", bufs=1) as wgt,
            tc.tile_pool(name="big", bufs=1) as big,
            tc.tile_pool(name="st_p", bufs=2, space="PSUM") as st_pool,
            tc.tile_pool(name="acc_p", bufs=1, space="PSUM") as acc_pool,
            tc.tile_pool(name="e_p", bufs=3) as e_pool,
            tc.tile_pool(name="misc", bufs=2) as misc,
        ):
            # weights on the ACT hwdge queue, x2 on sync, x2h on gpsimd
            wcat_s = wgt.tile([128, 772], F32R)
            nc.scalar.dma_start(out=wcat_s, in_=wcat[:, :])
            wv_s = wgt.tile([128, 2, 128], FP16)
            nc.scalar.dma_start(out=wv_s, in_=wv_r[:, :, :])
            wq_s = wcat_s[:, 0:256].rearrange("p (c m) -> p c m", c=2)
            wk_s = wcat_s[:, 256:512].rearrange("p (c m) -> p c m", c=2)
            wp_s = wcat_s[:, 512:768].rearrange("p (c m) -> p c m", c=2)
            tq_s = wcat_s[:, 768:769].bitcast(F32)
            tk_s = wcat_s[:, 769:770].bitcast(F32)
            bias_s = wcat_s[:, 770:772].bitcast(F32)

            x2_s = wgt.tile([128, 2, N], F32R)
            x2h_s = wgt.tile([128, 2, N], FP16)
            for a, b in xsplits:
                nc.sync.dma_start(out=x2_s[:, :, a:b], in_=x2[:, :, a:b])
                nc.gpsimd.dma_start(out=x2h_s[:, :, a:b], in_=x2h[:, :, a:b])

            ones_f = wgt.tile([128, 32], F32)
            nc.vector.memset(ones_f, 1.0)
            ones = wgt.tile([128, 32], FP16)
            nc.vector.tensor_copy(ones, ones_f)

            # --- q/k 1x1 convs (+ bias), replicated 2x on partitions; the
            # chunks stream in as the pipeline needs them.
            qt4 = big.tile([128, N], FP16)
            k4 = big.tile([128, N], FP16)

            def emit_conv_chunk(dst, w_s, t_s, n8):
                sl = slice(n8 * QC, (n8 + 1) * QC)
                cp = st_pool.tile([128, QC], F32, tag="st")
                nc.tensor.matmul(cp, w_s[:, 0, :], x2_s[:, 0, sl],
                                 start=True, stop=False)
                nc.tensor.matmul(cp, w_s[:, 1, :], x2_s[:, 1, sl],
                                 start=False, stop=True)
                nc.vector.tensor_scalar_add(dst[:, sl], cp, t_s[:, 0:1])

            emit_conv_chunk(qt4, wq_s, tq_s, 0)
            emit_conv_chunk(k4, wk_s, tk_s, 0)
            conv_done = {"k": 1}
            qt4_pending = list(range(1, NQC))

            # --- V tiles [m, dv] fp16, computed inside chunk 0's m-loop ---
            vt = big.tile([128, NMB, DV], FP16)

            def emit_vt_trio(group):
                vp = st_pool.tile([128, 3, DV], F32, tag="st")
                for i, mb in enumerate(group):
                    msl = slice(mb * 128, (mb + 1) * 128)
                    nc.tensor.matmul(vp[:, i, :], x2h_s[:, 0, msl],
                                     wv_s[:, 0, :], start=True, stop=False)
                    nc.tensor.matmul(vp[:, i, :], x2h_s[:, 1, msl],
                                     wv_s[:, 1, :], start=False, stop=True)
                ng = len(group)
                nc.vector.tensor_copy(
                    vt[:, group[0]:group[0] + ng, :].rearrange(
                        "p a b -> p (a b)"),
                    vp[:, 0:ng, :].rearrange("p a b -> p (a b)"))

            # --- main loop: per q-chunk, depth-2 software-pipelined trios ---
            groups = [list(range(g, min(g + 3, NMB))) for g in range(0, NMB, 3)]
            NG = len(groups)

            def emit_st(qsl, group):
                st = st_pool.tile([128, 3, QC], F32, tag="st")
                for i, mb in enumerate(group):
                    nc.tensor.matmul(
                        st[:, i, :],
                        k4[32 * i:32 * (i + 1), mb * 128:(mb + 1) * 128],
                        qt4[32 * i:32 * (i + 1), qsl],
                        start=True, stop=True,
                        tile_position=(32 * i, 0),
                    )
                return st

            pending_st = emit_st(slice(0, QC), groups[0])
            emit_vt_trio(groups[0])
            for qc in range(NQC):
                qsl = slice(qc * QC, (qc + 1) * QC)
                out1 = acc_pool.tile([128, QC], F32, tag="out1")
                cs = acc_pool.tile([128, QC], F32, tag="cs")
                es = [None] * NG

                def emit_pvcs(g):
                    group = groups[g]
                    e = es[g]
                    for i, mb in enumerate(group):
                        nc.tensor.matmul(
                            out1, vt[:, mb, :], e[:, i, :],
                            start=(mb == 0), stop=(mb == NMB - 1),
                            skip_group_check=True,
                        )
                    for i, mb in enumerate(group):
                        j = mb % 4
                        nc.tensor.matmul(
                            cs[32 * j:32 * (j + 1), :],
                            ones[:, 0:32], e[:, i, :],
                            start=(mb < 4), stop=(mb >= NMB - 4),
                            tile_position=(0, 32 * j),
                            skip_group_check=True,
                        )

                st_cur = pending_st
                for g, group in enumerate(groups):
                    ng = len(group)
                    e = e_pool.tile([128, 3, QC], FP16, tag="e")
                    es[g] = e
                    nc.scalar.activation(
                        out=e[:, 0:ng, :].rearrange("p a b -> p (a b)"),
                        in_=st_cur[:, 0:ng, :].rearrange("p a b -> p (a b)"),
                        func=EXP,
                    )
                    if qc == 0:
                        if g + 1 < NG:
                            emit_vt_trio(groups[g + 1])
                            need_k = min((384 * (g + 2) - 1) // QC + 1, NQC)
                            while conv_done["k"] < need_k:
                                emit_conv_chunk(k4, wk_s, tk_s, conv_done["k"])
                                conv_done["k"] += 1
                        if qt4_pending and g % 2 == 1:
                            emit_conv_chunk(qt4, wq_s, tq_s, qt4_pending.pop(0))
                    elif qt4_pending and g == 0:
                        emit_conv_chunk(qt4, wq_s, tq_s, qt4_pending.pop(0))
                    if g + 1 < NG:
                        st_cur = emit_st(qsl, groups[g + 1])
                    if g >= 1:
                        emit_pvcs(g - 1)
                emit_pvcs(NG - 1)
                if qc + 1 < NQC:
                    pending_st = emit_st(
                        slice((qc + 1) * QC, (qc + 2) * QC), groups[0])

                # epilogue: denominator; proj on RAW out1 (normalization
                # commutes through the per-q column scaling), early evac
                cs_s = misc.tile([128, QC], F32, tag="cs_s")
                nc.vector.tensor_copy(cs_s, cs)
                out1_s = misc.tile([128, QC], F32R, tag="out1_s")
                nc.vector.tensor_copy(out1_s, out1)
                zp = st_pool.tile([128, 2, QC], F32, tag="st")
                for ob in range(2):
                    nc.tensor.matmul(zp[:, ob, :], wp_s[:, ob, :], out1_s,
                                     start=True, stop=True)
                zraw = misc.tile([128, 2, QC], F32, tag="zraw")
                nc.vector.tensor_copy(
                    zraw.rearrange("p a b -> p (a b)"),
                    zp.rearrange("p a b -> p (a b)"))
                slab = misc.tile([32, 3, QC], F32, tag="slab")
                for j in range(3):
                    nc.sync.dma_start(out=slab[:, j, :],
                                      in_=cs_s[32 * (j + 1):32 * (j + 2), :])
                t1 = misc.tile([32, QC], F32, tag="t1")
                nc.vector.tensor_add(t1, cs_s[0:32, :], slab[:, 0, :])
                t2 = misc.tile([32, QC], F32, tag="t2")
                nc.vector.tensor_add(t2, slab[:, 1, :], slab[:, 2, :])
                tot = misc.tile([32, QC], F32, tag="tot")
                nc.vector.tensor_add(tot, t1, t2)
                recipb = misc.tile([128, QC], F32, tag="recipb")
                nc.vector.reciprocal(recipb[0:32, :], tot)
                for j in range(1, 4):
                    nc.sync.dma_start(out=recipb[32 * j:32 * (j + 1), :],
                                      in_=recipb[0:32, :])
                zs = misc.tile([128, 2, QC], F32, tag="zs")
                for ob in range(2):
                    nc.vector.tensor_mul(zs[:, ob, :], zraw[:, ob, :], recipb)
                    nc.vector.tensor_scalar_add(
                        zs[:, ob, :], zs[:, ob, :], bias_s[:, ob:ob + 1])
                    eng = (nc.sync, nc.gpsimd, nc.scalar)[(2 * qc + ob) % 3]
                    eng.dma_start(out=z[ob, :, qsl], in_=zs[:, ob, :])
    nc.compile()
    return nc


def _prepare_in_maps(x, wq, sq, tq, wk, sk, tk, wv, sv, tv, wp, sp, tp):
    X = np.ascontiguousarray(x.reshape(C, N), dtype=np.float32)
    x2 = np.ascontiguousarray(X.reshape(2, 128, N).transpose(1, 0, 2))
    x2h = x2.astype(np.float16)
    bias_full = (sp * (wp @ tv) + tp) / H  # [256], per-core share

    in_maps = []
    for h in range(H):
        wq_h = (wq[32 * h:32 * (h + 1), :] * sq[32 * h:32 * (h + 1), None])
        wk_h = (wk[32 * h:32 * (h + 1), :] * sk[32 * h:32 * (h + 1), None])
        wqT_rep = np.tile(wq_h.T, (1, 4))        # [256, 128]
        wkT_rep = np.tile(wk_h.T, (1, 4))
        wq_r = np.ascontiguousarray(
            wqT_rep.reshape(2, 128, 128).transpose(1, 0, 2), dtype=np.float32)
        wk_r = np.ascontiguousarray(
            wkT_rep.reshape(2, 128, 128).transpose(1, 0, 2), dtype=np.float32)
        tq4 = np.ascontiguousarray(
            np.tile(tq[32 * h:32 * (h + 1)], 4).reshape(128, 1), dtype=np.float32)
        tk4 = np.ascontiguousarray(
            np.tile(tk[32 * h:32 * (h + 1)], 4).reshape(128, 1), dtype=np.float32)
        wv_h = wv[128 * h:128 * (h + 1), :]      # [dv, c]
        wv_r = np.ascontiguousarray(
            wv_h.T.reshape(2, 128, 128).transpose(1, 0, 2)).astype(np.float16)
        Wp_h = (sp[:, None] * wp[:, 128 * h:128 * (h + 1)]
                * sv[None, 128 * h:128 * (h + 1)])   # [256, 128]
        wp_r = np.ascontiguousarray(
            Wp_h.reshape(2, 128, 128).transpose(2, 0, 1), dtype=np.float32)
        bias2 = np.ascontiguousarray(
            bias_full.reshape(2, 128).T, dtype=np.float32)
        wcat = np.concatenate([
            wq_r.reshape(128, 256), wk_r.reshape(128, 256),
            wp_r.reshape(128, 256), tq4, tk4, bias2,
        ], axis=1).astype(np.float32)
        in_maps.append({
            "x2": x2, "x2h": x2h, "wcat": np.ascontiguousarray(wcat),
            "wv_r": wv_r,
        })
    return in_maps


def kernel(**inputs):
    if "nc" not in _CACHED:
        _CACHED["nc"] = _build_program()
    nc = _CACHED["nc"]
    in_maps = _prepare_in_maps(**inputs)
    res = run_bass_kernel_spmd(nc, in_maps, core_ids=list(range(H)))
    y = np.zeros((2, 128, N), dtype=np.float64)
    for r in res.results:
        y += r["z"]
    return y.reshape(C, N).reshape(1, C, 64, 64).astype(np.float32)


# revision 16
# speedup vs baseline: 1.1073x; 1.1073x over previous
"""Trainium2 Bass kernel for nn_Attention (dense transformer attention block).

Reference computation (shapes hardcoded):
  x [1, 256, 64, 64]; 1x1 conv+BN to q/k/v; 8 heads; per-head
  softmax(q @ k) @ v over n=4096 positions (dqk=32, dv=128); 1x1 conv+BN proj.

Sharding: one head per NeuronCore (8 cores). Each core computes its head's
attention plus its partial contribution to the projection conv
(z_h = Wp_h @ out_h); the host sums the 8 partials (the only cross-head
coupling is the channel-concat feeding the proj conv, which is a sum of
per-head matmuls).

Folding tricks (host-side, exact):
 - BN scales sq/sk fold into wq/wk rows; biases tq/tk applied on-device
   per-partition.
 - v's BN affine (sv, tv) folds through the attention (softmax rows sum to 1):
   sv scales wp columns, tv contributes a constant bias via wp @ tv.
 - proj BN affine (sp, tp) folds into wp rows / the constant bias.

On-device layout avoids all transposes:
 - QT = [32, n] and K = [32, n] conv outputs (fp16), replicated 2x along
   partitions so S^T tiles come from 2-way row-packed (tile_position) matmuls.
 - S^T duo blocks [128 m, 2, 512 q] in PSUM -> one exp (ScalarE) -> E fp16.
 - V computed directly transposed ([m, dv] blocks, fp16) via lhsT=x.
 - PV accumulates over m in PSUM; softmax denominator via col-packed
   ones-matmul (colsum) of E; normalization + projection per q-chunk,
   fully software-pipelined (depth 2) so ScalarE exp runs back-to-back.

Matmul dtypes: float32r (full-rate fp32) for the q/k convs and the proj;
fp16 for S^T / PV / colsum / V. End-to-end absmax error vs fp64 reference
is ~1e-3 relative to output scale.
"""

import numpy as np

import concourse.bacc as bacc
import concourse.mybir as mybir
import concourse.tile as tile
from concourse.bass_utils import run_bass_kernel_spmd

F32 = mybir.dt.float32
F32R = mybir.dt.float32r
FP16 = mybir.dt.float16
EXP = mybir.ActivationFunctionType.Exp

N = 4096          # positions (64*64)
C = 256           # input channels
H = 8             # heads
DQK = 32
DV = 128
NMB = N // 128    # 32 m-blocks
NQC = 8           # q-chunks of 512
QC = 512

_CACHED = {}


def _build_program():
    nc = bacc.Bacc(trn_type="TRN2")

    x2 = nc.dram_tensor("x2", [128, 2, N], F32R, kind="ExternalInput")
    x2h = nc.dram_tensor("x2h", [128, 2, N], FP16, kind="ExternalInput")
    # all fp32 weights/biases packed into one tensor -> one startup DMA:
    # cols 0:256 wq (2 chunks), 256:512 wk, 512:768 wp, 768 tq, 769 tk,
    # 770:772 bias
    wcat = nc.dram_tensor("wcat", [128, 772], F32R, kind="ExternalInput")
    wv_r = nc.dram_tensor("wv_r", [128, 2, 128], FP16, kind="ExternalInput")
    z = nc.dram_tensor("z", [2, 128, N], F32, kind="ExternalOutput")

    # x pieces: small first piece so the first conv starts ASAP
    xsplits = [(0, 512), (512, 1536), (1536, 2560), (2560, 4096)]

    with tile.TileContext(nc) as tc:
        with (
            tc.tile_pool(name="wgt", bufs=1) as wgt,
            tc.tile_pool(name="big", bufs=1) as big,
            tc.tile_pool(name="st_p", bufs=2, space="PSUM") as st_pool,
            tc.tile_pool(name="acc_p", bufs=1, space="PSUM") as acc_pool,
            tc.tile_pool(name="e_p", bufs=3) as e_pool,
            tc.tile_pool(name="misc", bufs=2) as misc,
        ):
            # weights on the ACT hwdge queue, x2 on sync, x2h on gpsimd
            wcat_s = wgt.tile([128, 772], F32R)
            nc.scalar.dma_start(out=wcat_s, in_=wcat[:, :])
            wv_s = wgt.tile([128, 2, 128], FP16)
            nc.scalar.dma_start(out=wv_s, in_=wv_r[:, :, :])
            wq_s = wcat_s[:, 0:256].rearrange("p (c m) -> p c m", c=2)
            wk_s = wcat_s[:, 256:512].rearrange("p (c m) -> p c m", c=2)
            wp_s = wcat_s[:, 512:768].rearrange("p (c m) -> p c m", c=2)
            tq_s = wcat_s[:, 768:769].bitcast(F32)
            tk_s = wcat_s[:, 769:770].bitcast(F32)
            bias_s = wcat_s[:, 770:772].bitcast(F32)

            x2_s = wgt.tile([128, 2, N], F32R)
            x2h_s = wgt.tile([128, 2, N], FP16)
            for a, b in xsplits:
                nc.sync.dma_start(out=x2_s[:, :, a:b], in_=x2[:, :, a:b])
                nc.gpsimd.dma_start(out=x2h_s[:, :, a:b], in_=x2h[:, :, a:b])

            ones_f = wgt.tile([128, 32], F32)
            nc.vector.memset(ones_f, 1.0)
            ones = wgt.tile([128, 32], FP16)
            nc.vector.tensor_copy(ones, ones_f)

            # --- q/k 1x1 convs (+ bias), replicated 2x on partitions; the
            # chunks stream in as the pipeline needs them.
            qt4 = big.tile([128, N], FP16)
            k4 = big.tile([128, N], FP16)

            def emit_conv_chunk(dst, w_s, t_s, n8):
                sl = slice(n8 * QC, (n8 + 1) * QC)
                cp = st_pool.tile([128, QC], F32, tag="st")
                nc.tensor.matmul(cp, w_s[:, 0, :], x2_s[:, 0, sl],
                                 start=True, stop=False)
                nc.tensor.matmul(cp, w_s[:, 1, :], x2_s[:, 1, sl],
                                 start=False, stop=True)
                nc.vector.tensor_scalar_add(dst[:, sl], cp, t_s[:, 0:1])

            emit_conv_chunk(qt4, wq_s, tq_s, 0)
            emit_conv_chunk(k4, wk_s, tk_s, 0)
            conv_done = {"k": 1}
            qt4_pending = list(range(1, NQC))

            # --- V tiles [m, dv] fp16, computed inside chunk 0's m-loop ---
            vt = big.tile([128, NMB, DV], FP16)

            def emit_vt_trio(group):
                vp = st_pool.tile([128, 3, DV], F32, tag="st")
                for i, mb in enumerate(group):
                    msl = slice(mb * 128, (mb + 1) * 128)
                    nc.tensor.matmul(vp[:, i, :], x2h_s[:, 0, msl],
                                     wv_s[:, 0, :], start=True, stop=False)
                    nc.tensor.matmul(vp[:, i, :], x2h_s[:, 1, msl],
                                     wv_s[:, 1, :], start=False, stop=True)
                ng = len(group)
                nc.vector.tensor_copy(
                    vt[:, group[0]:group[0] + ng, :].rearrange(
                        "p a b -> p (a b)"),
                    vp[:, 0:ng, :].rearrange("p a b -> p (a b)"))

            # --- main loop: per q-chunk, depth-2 software-pipelined trios ---
            groups = [list(range(g, min(g + 3, NMB))) for g in range(0, NMB, 3)]
            NG = len(groups)
            out1n_all = big.tile([128, NQC, QC], F32R)

            def emit_st(qsl, group):
                st = st_pool.tile([128, 3, QC], F32, tag="st")
                for i, mb in enumerate(group):
                    nc.tensor.matmul(
                        st[:, i, :],
                        k4[32 * i:32 * (i + 1), mb * 128:(mb + 1) * 128],
                        qt4[32 * i:32 * (i + 1), qsl],
                        start=True, stop=True,
                        tile_position=(32 * i, 0),
                    )
                return st

            pending_st = emit_st(slice(0, QC), groups[0])
            emit_vt_trio(groups[0])
            for qc in range(NQC):
                qsl = slice(qc * QC, (qc + 1) * QC)
                out1 = acc_pool.tile([128, QC], F32, tag="out1")
                cs = acc_pool.tile([128, QC], F32, tag="cs")
                es = [None] * NG

                def emit_pvcs(g):
                    group = groups[g]
                    e = es[g]
                    for i, mb in enumerate(group):
                        nc.tensor.matmul(
                            out1, vt[:, mb, :], e[:, i, :],
                            start=(mb == 0), stop=(mb == NMB - 1),
                            skip_group_check=True,
                        )
                    for i, mb in enumerate(group):
                        j = mb % 4
                        nc.tensor.matmul(
                            cs[32 * j:32 * (j + 1), :],
                            ones[:, 0:32], e[:, i, :],
                            start=(mb < 4), stop=(mb >= NMB - 4),
                            tile_position=(0, 32 * j),
                            skip_group_check=True,
                        )

                st_cur = pending_st
                for g, group in enumerate(groups):
                    ng = len(group)
                    e = e_pool.tile([128, 3, QC], FP16, tag="e")
                    es[g] = e
                    nc.scalar.activation(
                        out=e[:, 0:ng, :].rearrange("p a b -> p (a b)"),
                        in_=st_cur[:, 0:ng, :].rearrange("p a b -> p (a b)"),
                        func=EXP,
                    )
                    if qc == 0:
                        if g + 1 < NG:
                            emit_vt_trio(groups[g + 1])
                            need_k = min((384 * (g + 2) - 1) // QC + 1, NQC)
                            while conv_done["k"] < need_k:
                                emit_conv_chunk(k4, wk_s, tk_s, conv_done["k"])
                                conv_done["k"] += 1
                        if qt4_pending and g % 2 == 1:
                            emit_conv_chunk(qt4, wq_s, tq_s, qt4_pending.pop(0))
                    elif qt4_pending and g == 0:
                        emit_conv_chunk(qt4, wq_s, tq_s, qt4_pending.pop(0))
                    if g + 1 < NG:
                        st_cur = emit_st(qsl, groups[g + 1])
                    if g >= 1:
                        emit_pvcs(g - 1)
                emit_pvcs(NG - 1)
                if qc + 1 < NQC:
                    pending_st = emit_st(
                        slice((qc + 1) * QC, (qc + 2) * QC), groups[0])

                # epilogue: softmax denominator + normalization (SBUF-side)
                cs_s = misc.tile([128, QC], F32, tag="cs_s")
                nc.vector.tensor_copy(cs_s, cs)
                out1_s = misc.tile([128, QC], F32, tag="out1_s")
                nc.vector.tensor_copy(out1_s, out1)
                slab = misc.tile([32, 3, QC], F32, tag="slab")
                for j in range(3):
                    nc.sync.dma_start(out=slab[:, j, :],
                                      in_=cs_s[32 * (j + 1):32 * (j + 2), :])
                t1 = misc.tile([32, QC], F32, tag="t1")
                nc.vector.tensor_add(t1, cs_s[0:32, :], slab[:, 0, :])
                t2 = misc.tile([32, QC], F32, tag="t2")
                nc.vector.tensor_add(t2, slab[:, 1, :], slab[:, 2, :])
                tot = misc.tile([32, QC], F32, tag="tot")
                nc.vector.tensor_add(tot, t1, t2)
                recipb = misc.tile([128, QC], F32, tag="recipb")
                nc.vector.reciprocal(recipb[0:32, :], tot)
                for j in range(1, 4):
                    nc.sync.dma_start(out=recipb[32 * j:32 * (j + 1), :],
                                      in_=recipb[0:32, :])
                nc.vector.tensor_mul(out1n_all[:, qc, :], out1_s, recipb)

            # --- projection tail: z[ob] = Wp_h[ob].T @ out1n (+bias) ---
            for qc in range(NQC):
                qsl = slice(qc * QC, (qc + 1) * QC)
                zp = st_pool.tile([128, 3, QC], F32, tag="st")
                for ob in range(2):
                    nc.tensor.matmul(zp[:, ob, :], wp_s[:, ob, :],
                                     out1n_all[:, qc, :],
                                     start=True, stop=True)
                zs = misc.tile([128, 2, QC], F32, tag="zs")
                for ob in range(2):
                    if qc % 2 == 0:
                        nc.vector.tensor_scalar_add(
                            zs[:, ob, :], zp[:, ob, :], bias_s[:, ob:ob + 1])
                    else:
                        nc.scalar.add(zs[:, ob, :], zp[:, ob, :],
                                      bias_s[:, ob:ob + 1])
                    eng = (nc.sync, nc.gpsimd, nc.scalar)[(2 * qc + ob) % 3]
                    eng.dma_start(out=z[ob, :, qsl], in_=zs[:, ob, :])
    nc.compile()
    return nc


def _prepare_in_maps(x, wq, sq, tq, wk, sk, tk, wv, sv, tv, wp, sp, tp):
    X = np.ascontiguousarray(x.reshape(C, N), dtype=np.float32)
    x2 = np.ascontiguousarray(X.reshape(2, 128, N).transpose(1, 0, 2))
    x2h = x2.astype(np.float16)
    bias_full = (sp * (wp @ tv) + tp) / H  # [256], per-core share

    in_maps = []
    for h in range(H):
        wq_h = (wq[32 * h:32 * (h + 1), :] * sq[32 * h:32 * (h + 1), None])
        wk_h = (wk[32 * h:32 * (h + 1), :] * sk[32 * h:32 * (h + 1), None])
        wqT_rep = np.tile(wq_h.T, (1, 4))        # [256, 128]
        wkT_rep = np.tile(wk_h.T, (1, 4))
        wq_r = np.ascontiguousarray(
            wqT_rep.reshape(2, 128, 128).transpose(1, 0, 2), dtype=np.float32)
        wk_r = np.ascontiguousarray(
            wkT_rep.reshape(2, 128, 128).transpose(1, 0, 2), dtype=np.float32)
        tq4 = np.ascontiguousarray(
            np.tile(tq[32 * h:32 * (h + 1)], 4).reshape(128, 1), dtype=np.float32)
        tk4 = np.ascontiguousarray(
            np.tile(tk[32 * h:32 * (h + 1)], 4).reshape(128, 1), dtype=np.float32)
        wv_h = wv[128 * h:128 * (h + 1), :]      # [dv, c]
        wv_r = np.ascontiguousarray(
            wv_h.T.reshape(2, 128, 128).transpose(1, 0, 2)).astype(np.float16)
        Wp_h = (sp[:, None] * wp[:, 128 * h:128 * (h + 1)]
                * sv[None, 128 * h:128 * (h + 1)])   # [256, 128]
        wp_r = np.ascontiguousarray(
            Wp_h.reshape(2, 128, 128).transpose(2, 0, 1), dtype=np.float32)
        bias2 = np.ascontiguousarray(
            bias_full.reshape(2, 128).T, dtype=np.float32)
        wcat = np.concatenate([
            wq_r.reshape(128, 256), wk_r.reshape(128, 256),
            wp_r.reshape(128, 256), tq4, tk4, bias2,
        ], axis=1).astype(np.float32)
        in_maps.append({
            "x2": x2, "x2h": x2h, "wcat": np.ascontiguousarray(wcat),
            "wv_r": wv_r,
        })
    return in_maps


def kernel(**inputs):
    if "nc" not in _CACHED:
        _CACHED["nc"] = _build_program()
    nc = _CACHED["nc"]
    in_maps = _prepare_in_maps(**inputs)
    res = run_bass_kernel_spmd(nc, in_maps, core_ids=list(range(H)))
    y = np.zeros((2, 128, N), dtype=np.float64)
    for r in res.results:
        y += r["z"]
    return y.reshape(C, N).reshape(1, C, 64, 64).astype(np.float32)


# revision 17
# speedup vs baseline: 1.1333x; 1.0235x over previous
"""Trainium2 Bass kernel for nn_Attention (dense transformer attention block).

Reference computation (shapes hardcoded):
  x [1, 256, 64, 64]; 1x1 conv+BN to q/k/v; 8 heads; per-head
  softmax(q @ k) @ v over n=4096 positions (dqk=32, dv=128); 1x1 conv+BN proj.

Sharding: one head per NeuronCore (8 cores). Each core computes its head's
attention plus its partial contribution to the projection conv
(z_h = Wp_h @ out_h); the host sums the 8 partials (the only cross-head
coupling is the channel-concat feeding the proj conv, which is a sum of
per-head matmuls).

Folding tricks (host-side, exact):
 - BN scales sq/sk fold into wq/wk rows; biases tq/tk applied on-device
   per-partition.
 - v's BN affine (sv, tv) folds through the attention (softmax rows sum to 1):
   sv scales wp columns, tv contributes a constant bias via wp @ tv.
 - proj BN affine (sp, tp) folds into wp rows / the constant bias.

On-device layout avoids all transposes:
 - QT = [32, n] and K = [32, n] conv outputs (fp16), replicated 2x along
   partitions so S^T tiles come from 2-way row-packed (tile_position) matmuls.
 - S^T duo blocks [128 m, 2, 512 q] in PSUM -> one exp (ScalarE) -> E fp16.
 - V computed directly transposed ([m, dv] blocks, fp16) via lhsT=x.
 - PV accumulates over m in PSUM; softmax denominator via col-packed
   ones-matmul (colsum) of E; normalization + projection per q-chunk,
   fully software-pipelined (depth 2) so ScalarE exp runs back-to-back.

Matmul dtypes: float32r (full-rate fp32) for the q/k convs and the proj;
fp16 for S^T / PV / colsum / V. End-to-end absmax error vs fp64 reference
is ~1e-3 relative to output scale.
"""

import numpy as np

import concourse.bacc as bacc
import concourse.mybir as mybir
import concourse.tile as tile
from concourse.bass_utils import run_bass_kernel_spmd

F32 = mybir.dt.float32
F32R = mybir.dt.float32r
FP16 = mybir.dt.float16
EXP = mybir.ActivationFunctionType.Exp

N = 4096          # positions (64*64)
C = 256           # input channels
H = 8             # heads
DQK = 32
DV = 128
NMB = N // 128    # 32 m-blocks
NQC = 8           # q-chunks of 512
QC = 512

_CACHED = {}


def _build_program():
    nc = bacc.Bacc(trn_type="TRN2")

    x2 = nc.dram_tensor("x2", [128, 2, N], F32R, kind="ExternalInput")
    x2h = nc.dram_tensor("x2h", [128, 2, N], FP16, kind="ExternalInput")
    # all fp32 weights/biases packed into one tensor -> one startup DMA:
    # cols 0:256 wq (2 chunks), 256:512 wk, 512:768 wp, 768 tq, 769 tk,
    # 770:772 bias
    wcat = nc.dram_tensor("wcat", [128, 772], F32R, kind="ExternalInput")
    wv_r = nc.dram_tensor("wv_r", [128, 2, 128], FP16, kind="ExternalInput")
    z = nc.dram_tensor("z", [2, 128, N], F32, kind="ExternalOutput")

    # x pieces: small first piece so the first conv starts ASAP
    xsplits = [(0, 512), (512, 1536), (1536, 2560), (2560, 4096)]

    with tile.TileContext(nc) as tc:
        with (
            tc.tile_pool(name="wgt", bufs=1) as wgt,
            tc.tile_pool(name="big", bufs=1) as big,
            tc.tile_pool(name="st_p", bufs=2, space="PSUM") as st_pool,
            tc.tile_pool(name="acc_p", bufs=1, space="PSUM") as acc_pool,
            tc.tile_pool(name="e_p", bufs=3) as e_pool,
            tc.tile_pool(name="misc", bufs=2) as misc,
        ):
            # weights on the ACT hwdge queue, x2 on sync, x2h on gpsimd
            wcat_s = wgt.tile([128, 772], F32R)
            nc.scalar.dma_start(out=wcat_s, in_=wcat[:, :])
            wv_s = wgt.tile([128, 2, 128], FP16)
            nc.scalar.dma_start(out=wv_s, in_=wv_r[:, :, :])
            wq_s = wcat_s[:, 0:256].rearrange("p (c m) -> p c m", c=2)
            wk_s = wcat_s[:, 256:512].rearrange("p (c m) -> p c m", c=2)
            wp_s = wcat_s[:, 512:768].rearrange("p (c m) -> p c m", c=2)
            tq_s = wcat_s[:, 768:769].bitcast(F32)
            tk_s = wcat_s[:, 769:770].bitcast(F32)
            bias_s = wcat_s[:, 770:772].bitcast(F32)

            x2_s = wgt.tile([128, 2, N], F32R)
            x2h_s = wgt.tile([128, 2, N], FP16)
            nc.sync.dma_start(out=x2_s[:, 0:1, 0:512], in_=x2[:, 0:1, 0:512])
            nc.gpsimd.dma_start(out=x2_s[:, 1:2, 0:512], in_=x2[:, 1:2, 0:512])
            for a, b in xsplits[1:]:
                nc.sync.dma_start(out=x2_s[:, :, a:b], in_=x2[:, :, a:b])
            for a, b in xsplits:
                nc.gpsimd.dma_start(out=x2h_s[:, :, a:b], in_=x2h[:, :, a:b])

            ones_f = wgt.tile([128, 32], F32)
            nc.vector.memset(ones_f, 1.0)
            ones = wgt.tile([128, 32], FP16)
            nc.vector.tensor_copy(ones, ones_f)

            # --- q/k 1x1 convs (+ bias), replicated 2x on partitions; the
            # chunks stream in as the pipeline needs them.
            qt4 = big.tile([128, N], FP16)
            k4 = big.tile([128, N], FP16)

            def emit_conv_chunk(dst, w_s, t_s, n8):
                sl = slice(n8 * QC, (n8 + 1) * QC)
                cp = st_pool.tile([128, QC], F32, tag="st")
                nc.tensor.matmul(cp, w_s[:, 0, :], x2_s[:, 0, sl],
                                 start=True, stop=False)
                nc.tensor.matmul(cp, w_s[:, 1, :], x2_s[:, 1, sl],
                                 start=False, stop=True)
                nc.vector.tensor_scalar_add(dst[:, sl], cp, t_s[:, 0:1])

            emit_conv_chunk(qt4, wq_s, tq_s, 0)
            emit_conv_chunk(k4, wk_s, tk_s, 0)
            conv_done = {"k": 1}
            qt4_pending = list(range(1, NQC))

            # --- V tiles [m, dv] fp16, computed inside chunk 0's m-loop ---
            vt = big.tile([128, NMB, DV], FP16)

            def emit_vt_trio(group):
                vp = st_pool.tile([128, 3, DV], F32, tag="st")
                for i, mb in enumerate(group):
                    msl = slice(mb * 128, (mb + 1) * 128)
                    nc.tensor.matmul(vp[:, i, :], x2h_s[:, 0, msl],
                                     wv_s[:, 0, :], start=True, stop=False)
                    nc.tensor.matmul(vp[:, i, :], x2h_s[:, 1, msl],
                                     wv_s[:, 1, :], start=False, stop=True)
                ng = len(group)
                nc.vector.tensor_copy(
                    vt[:, group[0]:group[0] + ng, :].rearrange(
                        "p a b -> p (a b)"),
                    vp[:, 0:ng, :].rearrange("p a b -> p (a b)"))

            # --- main loop: per q-chunk, depth-2 software-pipelined trios ---
            groups = [list(range(g, min(g + 3, NMB))) for g in range(0, NMB, 3)]
            NG = len(groups)

            def emit_st(qsl, group):
                st = st_pool.tile([128, 3, QC], F32, tag="st")
                for i, mb in enumerate(group):
                    nc.tensor.matmul(
                        st[:, i, :],
                        k4[32 * i:32 * (i + 1), mb * 128:(mb + 1) * 128],
                        qt4[32 * i:32 * (i + 1), qsl],
                        start=True, stop=True,
                        tile_position=(32 * i, 0),
                    )
                return st

            pending_st = emit_st(slice(0, QC), groups[0])
            emit_vt_trio(groups[0])
            for qc in range(NQC):
                qsl = slice(qc * QC, (qc + 1) * QC)
                out1 = acc_pool.tile([128, QC], F32, tag="out1")
                cs = acc_pool.tile([128, QC], F32, tag="cs")
                es = [None] * NG

                def emit_pvcs(g):
                    group = groups[g]
                    e = es[g]
                    for i, mb in enumerate(group):
                        nc.tensor.matmul(
                            out1, vt[:, mb, :], e[:, i, :],
                            start=(mb == 0), stop=(mb == NMB - 1),
                            skip_group_check=True,
                        )
                    for i, mb in enumerate(group):
                        j = mb % 4
                        nc.tensor.matmul(
                            cs[32 * j:32 * (j + 1), :],
                            ones[:, 0:32], e[:, i, :],
                            start=(mb < 4), stop=(mb >= NMB - 4),
                            tile_position=(0, 32 * j),
                            skip_group_check=True,
                        )

                st_cur = pending_st
                for g, group in enumerate(groups):
                    ng = len(group)
                    e = e_pool.tile([128, 3, QC], FP16, tag="e")
                    es[g] = e
                    nc.scalar.activation(
                        out=e[:, 0:ng, :].rearrange("p a b -> p (a b)"),
                        in_=st_cur[:, 0:ng, :].rearrange("p a b -> p (a b)"),
                        func=EXP,
                    )
                    if qc == 0:
                        if g + 1 < NG:
                            emit_vt_trio(groups[g + 1])
                            need_k = min((384 * (g + 2) - 1) // QC + 1, NQC)
                            while conv_done["k"] < need_k:
                                emit_conv_chunk(k4, wk_s, tk_s, conv_done["k"])
                                conv_done["k"] += 1
                        if qt4_pending and g % 2 == 1:
                            emit_conv_chunk(qt4, wq_s, tq_s, qt4_pending.pop(0))
                    elif qt4_pending and g == 0:
                        emit_conv_chunk(qt4, wq_s, tq_s, qt4_pending.pop(0))
                    if g + 1 < NG:
                        st_cur = emit_st(qsl, groups[g + 1])
                    elif qc + 1 < NQC:
                        pending_st = emit_st(
                            slice((qc + 1) * QC, (qc + 2) * QC), groups[0])
                    if g >= 1:
                        emit_pvcs(g - 1)
                emit_pvcs(NG - 1)

                # epilogue: proj on RAW out1 (per-q normalization commutes
                # through the channel contraction; applied to z below), with
                # zp borrowing the just-freed out1/cs psum slots.
                cs_s = misc.tile([128, QC], F32, tag="cs_s")
                nc.vector.tensor_copy(cs_s, cs)
                out1_s = misc.tile([128, QC], F32R, tag="out1_s")
                nc.vector.tensor_copy(out1_s, out1)
                zp0 = acc_pool.tile([128, QC], F32, tag="cs")
                zp1 = acc_pool.tile([128, QC], F32, tag="out1")
                nc.tensor.matmul(zp0, wp_s[:, 0, :], out1_s,
                                 start=True, stop=True)
                nc.tensor.matmul(zp1, wp_s[:, 1, :], out1_s,
                                 start=True, stop=True)
                zraw = misc.tile([128, 2, QC], F32, tag="zraw")
                nc.vector.tensor_copy(zraw[:, 0, :], zp0)
                nc.vector.tensor_copy(zraw[:, 1, :], zp1)
                slab = misc.tile([32, 3, QC], F32, tag="slab")
                for j in range(3):
                    nc.sync.dma_start(out=slab[:, j, :],
                                      in_=cs_s[32 * (j + 1):32 * (j + 2), :])
                t1 = misc.tile([32, QC], F32, tag="t1")
                nc.vector.tensor_add(t1, cs_s[0:32, :], slab[:, 0, :])
                t2 = misc.tile([32, QC], F32, tag="t2")
                nc.vector.tensor_add(t2, slab[:, 1, :], slab[:, 2, :])
                tot = misc.tile([32, QC], F32, tag="tot")
                nc.vector.tensor_add(tot, t1, t2)
                recipb = misc.tile([128, QC], F32, tag="recipb")
                nc.vector.reciprocal(recipb[0:32, :], tot)
                for j in range(1, 4):
                    nc.sync.dma_start(out=recipb[32 * j:32 * (j + 1), :],
                                      in_=recipb[0:32, :])
                zs = misc.tile([128, 2, QC], F32, tag="zs")
                for ob in range(2):
                    nc.vector.tensor_mul(zs[:, ob, :], zraw[:, ob, :], recipb)
                    nc.vector.tensor_scalar_add(
                        zs[:, ob, :], zs[:, ob, :], bias_s[:, ob:ob + 1])
                    eng = (nc.sync, nc.gpsimd)[(2 * qc + ob) % 2]
                    eng.dma_start(out=z[ob, :, qsl], in_=zs[:, ob, :])
    nc.compile()
    return nc


def _prepare_in_maps(x, wq, sq, tq, wk, sk, tk, wv, sv, tv, wp, sp, tp):
    X = np.ascontiguousarray(x.reshape(C, N), dtype=np.float32)
    x2 = np.ascontiguousarray(X.reshape(2, 128, N).transpose(1, 0, 2))
    x2h = x2.astype(np.float16)
    bias_full = (sp * (wp @ tv) + tp) / H  # [256], per-core share

    in_maps = []
    for h in range(H):
        wq_h = (wq[32 * h:32 * (h + 1), :] * sq[32 * h:32 * (h + 1), None])
        wk_h = (wk[32 * h:32 * (h + 1), :] * sk[32 * h:32 * (h + 1), None])
        wqT_rep = np.tile(wq_h.T, (1, 4))        # [256, 128]
        wkT_rep = np.tile(wk_h.T, (1, 4))
        wq_r = np.ascontiguousarray(
            wqT_rep.reshape(2, 128, 128).transpose(1, 0, 2), dtype=np.float32)
        wk_r = np.ascontiguousarray(
            wkT_rep.reshape(2, 128, 128).transpose(1, 0, 2), dtype=np.float32)
        tq4 = np.ascontiguousarray(
            np.tile(tq[32 * h:32 * (h + 1)], 4).reshape(128, 1), dtype=np.float32)
        tk4 = np.ascontiguousarray(
            np.tile(tk[32 * h:32 * (h + 1)], 4).reshape(128, 1), dtype=np.float32)
        wv_h = wv[128 * h:128 * (h + 1), :]      # [dv, c]
        wv_r = np.ascontiguousarray(
            wv_h.T.reshape(2, 128, 128).transpose(1, 0, 2)).astype(np.float16)
        Wp_h = (sp[:, None] * wp[:, 128 * h:128 * (h + 1)]
                * sv[None, 128 * h:128 * (h + 1)])   # [256, 128]
        wp_r = np.ascontiguousarray(
            Wp_h.reshape(2, 128, 128).transpose(2, 0, 1), dtype=np.float32)
        bias2 = np.ascontiguousarray(
            bias_full.reshape(2, 128).T, dtype=np.float32)
        wcat = np.concatenate([
            wq_r.reshape(128, 256), wk_r.reshape(128, 256),
            wp_r.reshape(128, 256), tq4, tk4, bias2,
        ], axis=1).astype(np.float32)
        in_maps.append({
            "x2": x2, "x2h": x2h, "wcat": np.ascontiguousarray(wcat),
            "wv_r": wv_r,
        })
    return in_maps


def kernel(**inputs):
    if "nc" not in _CACHED:
        _CACHED["nc"] = _build_program()
    nc = _CACHED["nc"]
    in_maps = _prepare_in_maps(**inputs)
    res = run_bass_kernel_spmd(nc, in_maps, core_ids=list(range(H)))
    y = np.zeros((2, 128, N), dtype=np.float64)
    for r in res.results:
        y += r["z"]
    return y.reshape(C, N).reshape(1, C, 64, 64).astype(np.float32)


# revision 18
# speedup vs baseline: 1.1578x; 1.0217x over previous
"""Trainium2 Bass kernel for nn_Attention (dense transformer attention block).

Reference computation (shapes hardcoded):
  x [1, 256, 64, 64]; 1x1 conv+BN to q/k/v; 8 heads; per-head
  softmax(q @ k) @ v over n=4096 positions (dqk=32, dv=128); 1x1 conv+BN proj.

Sharding: one head per NeuronCore (8 cores). Each core computes its head's
attention plus its partial contribution to the projection conv
(z_h = Wp_h @ out_h); the host sums the 8 partials (the only cross-head
coupling is the channel-concat feeding the proj conv, which is a sum of
per-head matmuls).

Folding tricks (host-side, exact):
 - BN scales sq/sk fold into wq/wk rows; biases tq/tk applied on-device
   per-partition.
 - v's BN affine (sv, tv) folds through the attention (softmax rows sum to 1):
   sv scales wp columns, tv contributes a constant bias via wp @ tv.
 - proj BN affine (sp, tp) folds into wp rows / the constant bias.

On-device layout avoids all transposes:
 - QT = [32, n] and K = [32, n] conv outputs (fp16), replicated 2x along
   partitions so S^T tiles come from 2-way row-packed (tile_position) matmuls.
 - S^T duo blocks [128 m, 2, 512 q] in PSUM -> one exp (ScalarE) -> E fp16.
 - V computed directly transposed ([m, dv] blocks, fp16) via lhsT=x.
 - PV accumulates over m in PSUM; softmax denominator via col-packed
   ones-matmul (colsum) of E; normalization + projection per q-chunk,
   fully software-pipelined (depth 2) so ScalarE exp runs back-to-back.

Matmul dtypes: float32r (full-rate fp32) for the q/k convs and the proj;
fp16 for S^T / PV / colsum / V. End-to-end absmax error vs fp64 reference
is ~1e-3 relative to output scale.
"""

import numpy as np

import concourse.bacc as bacc
import concourse.mybir as mybir
import concourse.tile as tile
from concourse.bass_utils import run_bass_kernel_spmd

F32 = mybir.dt.float32
F32R = mybir.dt.float32r
FP16 = mybir.dt.float16
EXP = mybir.ActivationFunctionType.Exp

N = 4096          # positions (64*64)
C = 256           # input channels
H = 8             # heads
DQK = 32
DV = 128
NMB = N // 128    # 32 m-blocks
NQC = 8           # q-chunks of 512
QC = 512

_CACHED = {}


def _build_program():
    nc = bacc.Bacc(trn_type="TRN2")

    x2 = nc.dram_tensor("x2", [128, 2, N], F32R, kind="ExternalInput")
    x2h = nc.dram_tensor("x2h", [128, 2, N], FP16, kind="ExternalInput")
    # all fp32 weights/biases packed into one tensor -> one startup DMA:
    # cols 0:256 wq (2 chunks), 256:512 wk, 512:768 wp, 768 tq, 769 tk,
    # 770:772 bias
    wcat = nc.dram_tensor("wcat", [128, 772], F32R, kind="ExternalInput")
    wv_r = nc.dram_tensor("wv_r", [128, 2, 128], FP16, kind="ExternalInput")
    z = nc.dram_tensor("z", [2, 128, N], F32, kind="ExternalOutput")

    # x pieces: small first piece so the first conv starts ASAP
    xsplits = [(0, 512), (512, 1536), (1536, 2560), (2560, 4096)]

    with tile.TileContext(nc) as tc:
        with (
            tc.tile_pool(name="wgt", bufs=1) as wgt,
            tc.tile_pool(name="big", bufs=1) as big,
            tc.tile_pool(name="st_p", bufs=2, space="PSUM") as st_pool,
            tc.tile_pool(name="acc_p", bufs=1, space="PSUM") as acc_pool,
            tc.tile_pool(name="e_p", bufs=3) as e_pool,
            tc.tile_pool(name="misc", bufs=2) as misc,
        ):
            # weights on the ACT hwdge queue, x2 on sync, x2h on gpsimd
            wcat_s = wgt.tile([128, 772], F32R)
            nc.scalar.dma_start(out=wcat_s, in_=wcat[:, :])
            wv_s = wgt.tile([128, 2, 128], FP16)
            nc.scalar.dma_start(out=wv_s, in_=wv_r[:, :, :])
            wq_s = wcat_s[:, 0:256].rearrange("p (c m) -> p c m", c=2)
            wk_s = wcat_s[:, 256:512].rearrange("p (c m) -> p c m", c=2)
            wp_s = wcat_s[:, 512:768].rearrange("p (c m) -> p c m", c=2)
            tq_s = wcat_s[:, 768:769].bitcast(F32)
            tk_s = wcat_s[:, 769:770].bitcast(F32)
            bias_s = wcat_s[:, 770:772].bitcast(F32)

            x2_s = wgt.tile([128, 2, N], F32R)
            x2h_s = wgt.tile([128, 2, N], FP16)
            # first conv piece split across two queues, then 512-col slices
            # of x2 (f32r, conv) and x2h (fp16, V) interleaved by need time
            nc.sync.dma_start(out=x2_s[:, 0:1, 0:512], in_=x2[:, 0:1, 0:512])
            nc.gpsimd.dma_start(out=x2_s[:, 1:2, 0:512], in_=x2[:, 1:2, 0:512])
            qs = [nc.sync, nc.gpsimd, nc.scalar]
            qi = 0
            for c in range(8):
                for t, d in ((x2_s, x2), (x2h_s, x2h)):
                    if t is x2_s and c == 0:
                        continue
                    sl = slice(c * 512, (c + 1) * 512)
                    qs[qi % 3].dma_start(out=t[:, :, sl], in_=d[:, :, sl])
                    qi += 1

            ones_f = wgt.tile([128, 32], F32)
            nc.vector.memset(ones_f, 1.0)
            ones = wgt.tile([128, 32], FP16)
            nc.vector.tensor_copy(ones, ones_f)

            # --- q/k 1x1 convs (+ bias), replicated 2x on partitions; the
            # chunks stream in as the pipeline needs them.
            qt4 = big.tile([128, N], FP16)
            k4 = big.tile([128, N], FP16)

            def emit_conv_chunk(dst, w_s, t_s, n8):
                sl = slice(n8 * QC, (n8 + 1) * QC)
                cp = st_pool.tile([128, QC], F32, tag="st")
                nc.tensor.matmul(cp, w_s[:, 0, :], x2_s[:, 0, sl],
                                 start=True, stop=False)
                nc.tensor.matmul(cp, w_s[:, 1, :], x2_s[:, 1, sl],
                                 start=False, stop=True)
                nc.vector.tensor_scalar_add(dst[:, sl], cp, t_s[:, 0:1])

            emit_conv_chunk(qt4, wq_s, tq_s, 0)
            emit_conv_chunk(k4, wk_s, tk_s, 0)
            conv_done = {"k": 1}
            qt4_pending = list(range(1, NQC))

            # --- V tiles [m, dv] fp16, computed inside chunk 0's m-loop ---
            vt = big.tile([128, NMB, DV], FP16)

            def emit_vt_trio(group):
                vp = st_pool.tile([128, 3, DV], F32, tag="st")
                for i, mb in enumerate(group):
                    msl = slice(mb * 128, (mb + 1) * 128)
                    nc.tensor.matmul(vp[:, i, :], x2h_s[:, 0, msl],
                                     wv_s[:, 0, :], start=True, stop=False)
                    nc.tensor.matmul(vp[:, i, :], x2h_s[:, 1, msl],
                                     wv_s[:, 1, :], start=False, stop=True)
                ng = len(group)
                nc.vector.tensor_copy(
                    vt[:, group[0]:group[0] + ng, :].rearrange(
                        "p a b -> p (a b)"),
                    vp[:, 0:ng, :].rearrange("p a b -> p (a b)"))

            # --- main loop: per q-chunk, depth-2 software-pipelined trios ---
            groups = [list(range(g, min(g + 3, NMB))) for g in range(0, NMB, 3)]
            NG = len(groups)

            def emit_st(qsl, group):
                st = st_pool.tile([128, 3, QC], F32, tag="st")
                for i, mb in enumerate(group):
                    nc.tensor.matmul(
                        st[:, i, :],
                        k4[32 * i:32 * (i + 1), mb * 128:(mb + 1) * 128],
                        qt4[32 * i:32 * (i + 1), qsl],
                        start=True, stop=True,
                        tile_position=(32 * i, 0),
                    )
                return st

            pending_st = emit_st(slice(0, QC), groups[0])
            emit_vt_trio(groups[0])
            for qc in range(NQC):
                qsl = slice(qc * QC, (qc + 1) * QC)
                out1 = acc_pool.tile([128, QC], F32, tag="out1")
                cs = acc_pool.tile([128, QC], F32, tag="cs")
                es = [None] * NG

                def emit_pvcs(g):
                    group = groups[g]
                    e = es[g]
                    for i, mb in enumerate(group):
                        nc.tensor.matmul(
                            out1, vt[:, mb, :], e[:, i, :],
                            start=(mb == 0), stop=(mb == NMB - 1),
                            skip_group_check=True,
                        )
                    for i, mb in enumerate(group):
                        j = mb % 4
                        nc.tensor.matmul(
                            cs[32 * j:32 * (j + 1), :],
                            ones[:, 0:32], e[:, i, :],
                            start=(mb < 4), stop=(mb >= NMB - 4),
                            tile_position=(0, 32 * j),
                            skip_group_check=True,
                        )

                st_cur = pending_st
                for g, group in enumerate(groups):
                    ng = len(group)
                    e = e_pool.tile([128, 3, QC], FP16, tag="e")
                    es[g] = e
                    nc.scalar.activation(
                        out=e[:, 0:ng, :].rearrange("p a b -> p (a b)"),
                        in_=st_cur[:, 0:ng, :].rearrange("p a b -> p (a b)"),
                        func=EXP,
                    )
                    if qc == 0 and g + 1 < NG:
                        need_k = min((384 * (g + 2) - 1) // QC + 1, NQC)
                        while conv_done["k"] < need_k:
                            emit_conv_chunk(k4, wk_s, tk_s, conv_done["k"])
                            conv_done["k"] += 1
                    if g + 1 < NG:
                        st_cur = emit_st(qsl, groups[g + 1])
                    elif qc + 1 < NQC:
                        pending_st = emit_st(
                            slice((qc + 1) * QC, (qc + 2) * QC), groups[0])
                    if g >= 1:
                        emit_pvcs(g - 1)
                    if qc == 0 and g + 1 < NG:
                        emit_vt_trio(groups[g + 1])
                    if qt4_pending and (qc == 0) == (g % 2 == 1) and \
                            (qc == 0 or g == 0):
                        emit_conv_chunk(qt4, wq_s, tq_s, qt4_pending.pop(0))
                emit_pvcs(NG - 1)

                # epilogue: proj on RAW out1 (per-q normalization commutes
                # through the channel contraction; applied to z below), with
                # zp borrowing the just-freed out1/cs psum slots.
                cs_s = misc.tile([128, QC], F32, tag="cs_s")
                nc.vector.tensor_copy(cs_s, cs)
                out1_s = misc.tile([128, QC], F32R, tag="out1_s")
                nc.vector.tensor_copy(out1_s, out1)
                zp0 = acc_pool.tile([128, QC], F32, tag="cs")
                zp1 = acc_pool.tile([128, QC], F32, tag="out1")
                nc.tensor.matmul(zp0, wp_s[:, 0, :], out1_s,
                                 start=True, stop=True)
                nc.tensor.matmul(zp1, wp_s[:, 1, :], out1_s,
                                 start=True, stop=True)
                zraw = misc.tile([128, 2, QC], F32, tag="zraw")
                nc.vector.tensor_copy(zraw[:, 0, :], zp0)
                nc.vector.tensor_copy(zraw[:, 1, :], zp1)
                slab = misc.tile([32, 3, QC], F32, tag="slab")
                for j in range(3):
                    nc.sync.dma_start(out=slab[:, j, :],
                                      in_=cs_s[32 * (j + 1):32 * (j + 2), :])
                t1 = misc.tile([32, QC], F32, tag="t1")
                nc.vector.tensor_add(t1, cs_s[0:32, :], slab[:, 0, :])
                t2 = misc.tile([32, QC], F32, tag="t2")
                nc.vector.tensor_add(t2, slab[:, 1, :], slab[:, 2, :])
                tot = misc.tile([32, QC], F32, tag="tot")
                nc.vector.tensor_add(tot, t1, t2)
                recipb = misc.tile([128, QC], F32, tag="recipb")
                nc.vector.reciprocal(recipb[0:32, :], tot)
                for j in range(1, 4):
                    nc.sync.dma_start(out=recipb[32 * j:32 * (j + 1), :],
                                      in_=recipb[0:32, :])
                zs = misc.tile([128, 2, QC], F32, tag="zs")
                for ob in range(2):
                    nc.vector.tensor_mul(zs[:, ob, :], zraw[:, ob, :], recipb)
                    nc.vector.tensor_scalar_add(
                        zs[:, ob, :], zs[:, ob, :], bias_s[:, ob:ob + 1])
                    eng = (nc.sync, nc.gpsimd)[(2 * qc + ob) % 2]
                    eng.dma_start(out=z[ob, :, qsl], in_=zs[:, ob, :])
    nc.compile()
    return nc


def _prepare_in_maps(x, wq, sq, tq, wk, sk, tk, wv, sv, tv, wp, sp, tp):
    X = np.ascontiguousarray(x.reshape(C, N), dtype=np.float32)
    x2 = np.ascontiguousarray(X.reshape(2, 128, N).transpose(1, 0, 2))
    x2h = x2.astype(np.float16)
    bias_full = (sp * (wp @ tv) + tp) / H  # [256], per-core share

    in_maps = []
    for h in range(H):
        wq_h = (wq[32 * h:32 * (h + 1), :] * sq[32 * h:32 * (h + 1), None])
        wk_h = (wk[32 * h:32 * (h + 1), :] * sk[32 * h:32 * (h + 1), None])
        wqT_rep = np.tile(wq_h.T, (1, 4))        # [256, 128]
        wkT_rep = np.tile(wk_h.T, (1, 4))
        wq_r = np.ascontiguousarray(
            wqT_rep.reshape(2, 128, 128).transpose(1, 0, 2), dtype=np.float32)
        wk_r = np.ascontiguousarray(
            wkT_rep.reshape(2, 128, 128).transpose(1, 0, 2), dtype=np.float32)
        tq4 = np.ascontiguousarray(
            np.tile(tq[32 * h:32 * (h + 1)], 4).reshape(128, 1), dtype=np.float32)
        tk4 = np.ascontiguousarray(
            np.tile(tk[32 * h:32 * (h + 1)], 4).reshape(128, 1), dtype=np.float32)
        wv_h = wv[128 * h:128 * (h + 1), :]      # [dv, c]
        wv_r = np.ascontiguousarray(
            wv_h.T.reshape(2, 128, 128).transpose(1, 0, 2)).astype(np.float16)
        Wp_h = (sp[:, None] * wp[:, 128 * h:128 * (h + 1)]
                * sv[None, 128 * h:128 * (h + 1)])   # [256, 128]
        wp_r = np.ascontiguousarray(
            Wp_h.reshape(2, 128, 128).transpose(2, 0, 1), dtype=np.float32)
        bias2 = np.ascontiguousarray(
            bias_full.reshape(2, 128).T, dtype=np.float32)
        wcat = np.concatenate([
            wq_r.reshape(128, 256), wk_r.reshape(128, 256),
            wp_r.reshape(128, 256), tq4, tk4, bias2,
        ], axis=1).astype(np.float32)
        in_maps.append({
            "x2": x2, "x2h": x2h, "wcat": np.ascontiguousarray(wcat),
            "wv_r": wv_r,
        })
    return in_maps


def kernel(**inputs):
    if "nc" not in _CACHED:
        _CACHED["nc"] = _build_program()
    nc = _CACHED["nc"]
    in_maps = _prepare_in_maps(**inputs)
    res = run_bass_kernel_spmd(nc, in_maps, core_ids=list(range(H)))
    y = np.zeros((2, 128, N), dtype=np.float64)
    for r in res.results:
        y += r["z"]
    return y.reshape(C, N).reshape(1, C, 64, 64).astype(np.float32)
